# revision 43
# baseline (speedup 1.0000x reference)
"""Trainium2 Bass kernel for decomposed-rel-pos attention (B=4, H=W=32, DIM=768, HEADS=12).

Sharding: 48 (batch, head) pairs -> 8 cores x 6 heads (core c: batch c//2,
heads (c%2)*6 .. +6). All matmul operands bf16 (fp32 PSUM accumulation),
bf16 output partials summed on host in fp32.

v2 schedule: engines execute in program order, so emission order IS the
schedule. PE is the binding engine (~67us of column-streaming); the program
is laid out so PE never head-of-line blocks:
 - per-head ladder of 8 slots; each slot emits the next S matmul pair first,
   then bounded filler (qk 3mm/slot over slots 0-3, AV units lag 3, the last
   three AV units of head h run in slots 0-2 of ladder h+1 so the tail burst
   never delays the next head's S).
 - the qk->extract->rel->rel-copy chain for head h+1 runs inside ladder h:
   extract-k on ACT (its slack engine), extract-q/rel copies on DVE,
   exp(h,4) on DVE via int16-Schraudolph bitcast (rel err ~2% on 1/8 of
   attn mass) to keep ACT <= PE.
 - prologue: qk(0) matmuls emitted per k-tile so they pipeline behind the
   per-tile x DMA; S(0,0) is split into k-contraction + bias-contraction
   halves so exp0 starts before the rel copies complete.
 - V projections (all 8 token-tiles) fill ladder-0/1 slots, each just ahead
   of its AV deadline.
 - epilogue: proj m-tiles on 4 rotating psum pools right after the last AV
   snapshot; output copies split ACT||DVE; out DMA alternates two queues.
"""
from contextlib import ExitStack

import numpy as np
import ml_dtypes

import concourse.bass as bass
import concourse.bacc as bacc
import concourse.mybir as mybir
import concourse.tile as tile
from concourse.bass_utils import run_bass_kernel_spmd

B, H, W, DIM, HEADS = 4, 32, 32, 768, 12
HD = DIM // HEADS  # 64
N = H * W  # 1024
HPC = HEADS // 2  # heads per core = 6
NCORES = 8
F32 = mybir.dt.float32
BF16 = mybir.dt.bfloat16
I16 = mybir.dt.int16
EXPF = mybir.ActivationFunctionType.Exp
IDENT = mybir.ActivationFunctionType.Identity
MUL = mybir.AluOpType.mult
ADD = mybir.AluOpType.add

# Schraudolph bf16 exp: i16 = round(x * 2^7/ln2 + (127*128 - 5.5)); bits are
# the bf16 value of ~exp(x) (max rel err 3.3%). Used for one of 8 exp tiles
# per head to offload ACT.
A16 = float(2 ** 7 / np.log(2))
B16 = float(127 * 128 - 5.5)

_cache = {}
PHASES = []  # (label, next_instruction_number) — filled during build for tracing


def _mark(nc, label):
    PHASES.append((label, int(nc.get_next_instruction_name()[2:])))


def build_program(reps=1):
    nc = bacc.Bacc("TRN2", target_bir_lowering=False, debug=False,
                   enable_asserts=False, num_devices=NCORES)
    xT = nc.dram_tensor("xT", [DIM, N], BF16, kind="ExternalInput")
    wqk = nc.dram_tensor("wqk", [DIM, HPC * 128], BF16, kind="ExternalInput")
    wv = nc.dram_tensor("wv", [DIM, HPC * 64], BF16, kind="ExternalInput")
    wp = nc.dram_tensor("wp", [HPC * HD, DIM], BF16, kind="ExternalInput")
    rhT = nc.dram_tensor("rhT", [HD, N], BF16, kind="ExternalInput")
    rwT = nc.dram_tensor("rwT", [HD, N], BF16, kind="ExternalInput")
    ecomb = nc.dram_tensor("ecomb", [64, N], BF16, kind="ExternalInput")
    qkb = nc.dram_tensor("qkb", [128, HPC], F32, kind="ExternalInput")
    ident = nc.dram_tensor("ident", [128, 128], BF16, kind="ExternalInput")
    out_d = nc.dram_tensor("out_part", [N, DIM], BF16, kind="ExternalOutput")

    with ExitStack() as ctx:
        tc = ctx.enter_context(tile.TileContext(nc))
        _body(nc, tc, ctx, xT, wqk, wv, wp, rhT, rwT, ecomb, qkb, ident, out_d)
    nc.compile()
    return nc


def _body(nc, tc, ctx, xT, wqk, wv, wp, rhT, rwT, ecomb, qkb, ident, out_d):
    persist = ctx.enter_context(tc.tile_pool(name="persist", bufs=1))
    small = ctx.enter_context(tc.tile_pool(name="small", bufs=3))
    outp = ctx.enter_context(tc.tile_pool(name="outp", bufs=8))
    ps_s = ctx.enter_context(tc.tile_pool(name="ps_s", bufs=2, space="PSUM"))
    ps_qk = ctx.enter_context(tc.tile_pool(name="ps_qk", bufs=1, space="PSUM"))
    ps_av = ctx.enter_context(tc.tile_pool(name="ps_av", bufs=1, space="PSUM"))

    # ---- static SBUF tiles ----
    xt_sb = persist.tile([128, 6, N], BF16, tag="xt", name="xt")
    wqk_sb = persist.tile([128, 6, HPC * 128], BF16, tag="wqk", name="wqk")
    wv_sb = persist.tile([128, 6, HPC * 64], BF16, tag="wv", name="wv")
    wp_sb = persist.tile([128, 3, DIM], BF16, tag="wp", name="wp")
    rhT_sb = persist.tile([HD, N], BF16, tag="rhT", name="rhT")
    rwT_sb = persist.tile([HD, N], BF16, tag="rwT", name="rwT")
    qkb_sb = persist.tile([128, HPC], F32, tag="qkb", name="qkb")
    lhsT = [persist.tile([128, N], BF16, tag=f"lhsT{p}", name=f"lhsT{p}") for p in range(2)]
    rhs_c = [persist.tile([128, N], BF16, tag=f"rhs{p}", name=f"rhs{p}") for p in range(2)]
    v_sb = [persist.tile([128, HPC * 65], BF16, tag=f"v{m}", name=f"v{m}") for m in range(8)]
    attnT = [persist.tile([128, N], BF16, tag=f"attnT{kb}", name=f"attnT{kb}") for kb in range(8)]
    attn_out = [persist.tile([128, N], BF16, tag=f"ao{j}", name=f"ao{j}") for j in range(3)]
    proj_lhsT = [persist.tile([128, N], BF16, tag=f"pl{j}", name=f"pl{j}") for j in range(3)]
    ident_sb = persist.tile([128, 128], BF16, tag="ident", name="ident")
    partial_sb = persist.tile([128, 8, DIM], BF16, tag="partial", name="partial")

    # ---- input DMA ----
    # The HWDGE descriptor generator is a single shared serial device
    # (~630ns per dma_start) and transfers serialize on the DMA bus, so:
    # few instructions, ordered by consumption deadline. x in 3 two-ktile
    # chunks so qk0 pipelines behind the bus; everything qk0/S(0,*) needs
    # (wqk head-0 cols, qkb, rel tables, ecomb) lands before wv/wqk-rest/wp.
    xt3 = xT.rearrange("(kt p) c -> p kt c", p=128)
    wqk3 = wqk.rearrange("(kt p) c -> p kt c", p=128)
    nc.sync.dma_start(xt_sb[:, 0:2, :], xt3[:, 0:2, :])
    nc.scalar.dma_start(wqk_sb[:, :, 0:128], wqk3[:, :, 0:128])
    nc.sync.dma_start(xt_sb[:, 2:4, :], xt3[:, 2:4, :])
    nc.scalar.dma_start(qkb_sb[:], qkb[:])
    nc.sync.dma_start(xt_sb[:, 4:6, :], xt3[:, 4:6, :])
    nc.scalar.dma_start(rhT_sb[:], rhT[:])
    nc.scalar.dma_start(rwT_sb[:], rwT[:])
    nc.scalar.dma_start(rhs_c[0][64:128, :], ecomb[:])
    nc.scalar.dma_start(rhs_c[1][64:128, :], ecomb[:])
    nc.sync.dma_start(wqk_sb[:, :, 128:256], wqk3[:, :, 128:256])
    nc.sync.dma_start(wv_sb[:], wv.rearrange("(kt p) c -> p kt c", p=128))
    nc.sync.dma_start(wqk_sb[:, :, 256:768], wqk3[:, :, 256:768])
    nc.scalar.dma_start(wp_sb[:], wp.rearrange("(t p) c -> p t c", p=128))
    nc.scalar.dma_start(ident_sb[:], ident[:])

    # ones columns of V (row-sum trick), written once
    for m in range(8):
        v3 = v_sb[m][:].rearrange("p (h c) -> p h c", c=65)
        nc.gpsimd.memset(v3[:, :, 64], 1.0)

    # prime the ACT table load (~1.3us) off the critical path at t~0
    dummy = persist.tile([1, 8], F32, tag="dummy", name="dummy")
    nc.gpsimd.memset(dummy[0:1, 0:4], 0.0)
    nc.scalar.activation(dummy[0:1, 4:8], dummy[0:1, 0:4], EXPF)
    # PE warm-up source (zeros): dummy matmuls keep the PE p-state ramp going
    # through the x-DMA window so qk0 runs at full clock.
    warm_sb = persist.tile([128, 512], BF16, tag="warm", name="warm")
    nc.gpsimd.memset(warm_sb[:], 0.0)
    warm_ps = ps_av.tile([128, 512], F32, tag="pav", name="warm_ps")

    def warm(n):
        for _ in range(n):
            nc.tensor.matmul(warm_ps[:], warm_sb[:, 0:128], warm_sb[:],
                             start=True, stop=True)


    # ---- phase helpers ----
    state = {}

    def qk_ktile(h, kt):
        """qk projection for head h, one k-tile (2 matmuls)."""
        if kt == 0:
            _mark(nc, f"qk{h}")
            state["pq"] = ps_qk.tile([128, N], F32, tag="pqk", name="pqk")
        pq = state["pq"]
        for half in range(2):
            sl = slice(half * 512, half * 512 + 512)
            nc.tensor.matmul(pq[:, sl], wqk_sb[:, kt, h * 128:(h + 1) * 128],
                             xt_sb[:, kt, sl], start=(kt == 0), stop=(kt == 5))
        return pq

    def phase_extract(h):
        """Both extracts on DVE: the ACT queue must stay pure exps — any
        insert there stretches the S<->exp psum-rotation loop (the period).
        The chain has a full ladder of lead time now, so DVE-serial is fine."""
        _mark(nc, f"extract{h}")
        pq, p = state["pq"], h % 2
        nc.vector.tensor_scalar(lhsT[p][0:64, :], pq[0:64, :],
                                0.125, qkb_sb[0:64, h:h + 1], MUL, ADD)
        nc.vector.tensor_scalar(rhs_c[p][0:64, :], pq[64:128, :],
                                qkb_sb[64:128, h:h + 1], None, ADD)

    def phase_rel_h(h):
        _mark(nc, f"relh{h}")
        pq, p = state["pq"], h % 2
        prh = pq[0:32, :]
        for qh in range(32):
            sl = slice(qh * 32, qh * 32 + 32)
            nc.tensor.matmul(prh[:, sl], rhT_sb[:, sl], lhsT[p][0:64, sl],
                             start=True, stop=True)

    def phase_rel_w(h):
        _mark(nc, f"relw{h}")
        pq, p = state["pq"], h % 2
        prw = pq[32:64, :]
        qT3 = lhsT[p][0:64, :].rearrange("p (a b) -> p b a", b=32)
        for qw in range(32):
            sl = slice(qw * 32, qw * 32 + 32)
            nc.tensor.matmul(prw[:, sl], rwT_sb[:, sl], qT3[:, qw, :],
                             start=True, stop=True)

    def phase_rel(h):
        phase_rel_h(h)
        phase_rel_w(h)

    def phase_relch(h):
        _mark(nc, f"relch{h}")
        return nc.vector.tensor_copy(lhsT[h % 2][64:96, :], state["pq"][0:32, :])

    def phase_relcw(h, ch_inst=None):
        _mark(nc, f"relcw{h}")
        prw_v = state["pq"][32:64, :].rearrange("p (a b) -> p b a", b=32)
        cw = nc.vector.tensor_copy(lhsT[h % 2][96:128, :], prw_v[:, :, :])
        if ch_inst is not None:
            # Tile adds a false WW dep between the two rel copies (disjoint
            # partition ranges of the same tile); drop it so they run in
            # parallel on ACT/DVE — this pair is on the handover chain.
            cw.ins.try_remove_dependency(ch_inst.ins.name)
        return cw

    def v_mm(m):
        _mark(nc, f"vmm{m}")
        pv = ps_s.tile([128, HPC * 64], F32, tag="ps", name="pv")
        state[f"pv{m}"] = pv
        for kt in range(6):
            nc.tensor.matmul(pv[:], xt_sb[:, kt, m * 128:(m + 1) * 128],
                             wv_sb[:, kt, :], start=(kt == 0), stop=(kt == 5))

    def v_copy(m):
        _mark(nc, f"vcp{m}")
        pv = state.pop(f"pv{m}")
        dst = v_sb[m][:].rearrange("p (h c) -> p h c", c=65)[:, :, 0:64]
        nc.vector.tensor_copy(dst, pv[:].rearrange("p (h c) -> p h c", c=64))

    def S_unit(h, kb, dve_exp=False, defer_exp=False):
        _mark(nc, f"S{h}.{kb}")
        p = h % 2
        ps = ps_s.tile([128, N], F32, tag="ps", name="s_ps")
        for half in range(2):
            sl = slice(half * 512, half * 512 + 512)
            nc.tensor.matmul(ps[:, sl], rhs_c[p][:, kb * 128:(kb + 1) * 128],
                             lhsT[p][:, sl], start=True, stop=True)
        if defer_exp:
            return lambda: _exp(kb, ps, dve_exp)
        _exp(kb, ps, dve_exp)

    def _exp(kb, ps, dve_exp):
        if dve_exp:
            nc.vector.tensor_scalar(attnT[kb][:].bitcast(I16), ps[:],
                                    A16, B16, MUL, ADD)
        else:
            nc.scalar.activation(attnT[kb][:], ps[:], EXPF)

    # AV psum layout: qb 0-3 at cols qb*65 (bank 0), qb 4-7 at 512+(qb-4)*65
    # (bank 1); accumulation start/stop is bank-granular.
    def avcol(qb):
        return qb * 65 if qb < 4 else 512 + (qb - 4) * 65

    def AV_unit(h, kb):
        _mark(nc, f"AV{h}.{kb}")
        if kb == 0:
            state["pav"] = ps_av.tile([128, 512 + 4 * 65], F32, tag="pav", name="pav")
        pav = state["pav"]
        for qb in range(8):
            c = avcol(qb)
            nc.tensor.matmul(pav[:, c:c + 65],
                             attnT[kb][:, qb * 128:(qb + 1) * 128],
                             v_sb[kb][:, h * 65:(h + 1) * 65],
                             start=(kb == 0 and qb % 4 == 0),
                             stop=(kb == 7 and qb % 4 == 3))

    def transp(j, half=None):
        _mark(nc, f"transp{j}")
        pl4 = proj_lhsT[j][:].rearrange("p (m t) -> p m t", t=128)
        if half is None:
            nc.sync.dma_start_transpose(pl4, attn_out[j][:])
        else:
            q0, q1 = [(0, 2), (2, 4), (4, 8)][half]
            nc.sync.dma_start_transpose(pl4[:, q0:q1, :],
                                        attn_out[j][:, q0 * 128:q1 * 128])

    def finish_head(h, last=False):
        """Denominators, reciprocal, AV snapshot, normalization, transpose.
        Runs after AV(h,7); for h<5 this is inside ladder h+1 (slot 2+)."""
        _mark(nc, f"fin{h}")
        pav = state.pop("pav")
        denom = small.tile([128, 8], F32, tag="denom", name="denom")
        recip = small.tile([128, 8], F32, tag="recip", name="recip")
        pva = pav[:, 0:260].rearrange("p (a b) -> p a b", b=65)
        pvb = pav[:, 512:772].rearrange("p (a b) -> p a b", b=65)
        nc.vector.tensor_copy(denom[:, 0:4], pva[:, :, 64])
        nc.vector.tensor_copy(denom[:, 4:8], pvb[:, :, 64])
        nc.vector.reciprocal_approx_fast(out=recip[:], in_=denom[:])

        pavs = small.tile([128, 520], F32, tag="pavs", name="pavs")
        # snapshot releases the AV psum; the last head splits it ACT||DVE
        # (ACT is idle after the final exp).
        (nc.scalar.copy if last else nc.vector.tensor_copy)(
            pavs[:, 260:520], pav[:, 512:772])
        nc.vector.tensor_copy(pavs[:, 0:260], pav[:, 0:260])

        def dst_of(qb):
            return attn_out[h // 2][:, qb * 128 + (h % 2) * 64:
                                    qb * 128 + (h % 2) * 64 + 64]

        for qb in range(8):
            eng = nc.vector if qb % 2 == 0 else nc.gpsimd
            eng.tensor_scalar(dst_of(qb), pavs[:, qb * 65:qb * 65 + 64],
                              recip[:, qb:qb + 1], None, MUL)
            if last and qb == 1:
                transp(2, half=0)
            elif last and qb == 3:
                transp(2, half=1)
        if last:
            transp(2, half=2)
        elif h == 1:
            transp(0)
        elif h == 3:
            transp(1)

    # ---- prologue: head 0 ----
    # qk(0) per k-tile pipelines behind the x-chunk DMAs; warm-up matmuls
    # fill the DMA waits and keep the PE clock ramped.
    for kt in range(6):
        pq = qk_ktile(0, kt)
        if kt % 2 == 1:
            warm(4)
    _mark(nc, "extract0")
    # parallel extract: k rows on ACT (idle until first exp), q rows on DVE
    nc.scalar.activation(rhs_c[0][0:64, :], pq[64:128, :], IDENT,
                         bias=qkb_sb[64:128, 0:1])
    nc.vector.tensor_scalar(lhsT[0][0:64, :], pq[0:64, :],
                            0.125, qkb_sb[0:64, 0:1], MUL, ADD)
    phase_rel(0)
    # S(0,0) split into k-contraction half (ready after extracts) and bias
    # half (after rel copies) so exp0 starts ~1us earlier.
    _mark(nc, "S0.0")
    ps00 = ps_s.tile([128, N], F32, tag="ps", name="s_ps")
    for half in range(2):
        sl = slice(half * 512, half * 512 + 512)
        nc.tensor.matmul(ps00[:, sl], rhs_c[0][0:64, 0:128],
                         lhsT[0][0:64, sl], start=True, stop=False)
    _mark(nc, "relc0")
    ch0 = nc.scalar.copy(lhsT[0][64:96, :], pq[0:32, :])
    phase_relcw(0, ch0)
    _mark(nc, "S0.0b")
    for half in range(2):
        sl = slice(half * 512, half * 512 + 512)
        nc.tensor.matmul(ps00[:, sl], rhs_c[0][64:128, 0:128],
                         lhsT[0][64:128, sl], start=False, stop=True)
    nc.scalar.activation(attnT[0][:], ps00[:], EXPF)

    # ---- ladders ----
    # Slot plan for ladder h (steady state h>=1, nh=h+1 prepared):
    #   slot 0..2: S(h,1..3) + AV(h-1,5..7) + qk(nh) kt 2k,2k+1... (3mm/slot
    #              over slots 0-3) ; finish_head(h-1) after AV(h-1,7)
    #   slot 3:    S(h,4) + AV(h,0) + qk rest + extract(nh)
    #   slot 4:    S(h,5) + AV(h,1)   [S(h,4) exp on DVE]
    #   slot 5:    S(h,6) + AV(h,2) + rel(nh)
    #   slot 6:    S(h,7) + AV(h,3) + relch/relcw(nh)
    #   slot 7:    AV(h,4) + S(nh,0)
    # Ladder 0 additionally carries the 8 V projections (deadline: AV(0,m)).
    # Steady-state chain pipelining: qk for head h+2 is emitted in slots 5-7
    # of ladder h (the qk psum frees once head h+1's rel copies read it at
    # slot 2), so when ladder h+1 starts, extract(h+2) can run immediately at
    # slot 0, rel at slot 1, rel copies at slot 2 — the handover chain
    # completes ~4 slots before S(h+2, 0) needs it.
    for h in range(HPC):
        nh, nh2 = h + 1, h + 2
        have_next = nh < HPC

        # slot 0
        if h == 0:
            v_mm(0)
            v_copy(0)
            v_mm(1)
            v_copy(1)
            for kt in range(6):
                qk_ktile(1, kt)  # one-time burst; ladder-0 slots are light
        else:
            S_unit(h, 1)
            AV_unit(h - 1, 5)
            if have_next:
                phase_extract(nh)

        # slot 1
        if h == 0:
            S_unit(0, 1)
            v_mm(2)
            v_copy(2)
            v_mm(3)
            v_copy(3)
            phase_extract(1)
        else:
            S_unit(h, 2)
            AV_unit(h - 1, 6)
            if have_next:
                phase_rel_h(nh)

        # slot 2
        if h == 0:
            S_unit(0, 2)
            v_mm(4)
            v_copy(4)
            v_mm(5)
            v_copy(5)
            S_unit(0, 3)
        else:
            S_unit(h, 3)
            AV_unit(h - 1, 7)
            if have_next:
                phase_rel_w(nh)
                ch = phase_relch(nh)
                phase_relcw(nh, ch)

        # slot 3  (qk(nh2) may only start after relc(nh) is emitted: the qk
        # psum WAR dep must point at already-emitted readers — so in ladder 0,
        # where relc(1) lands at slot 4, qk(2) waits until slot 5).
        # finish_head(h-1) is emitted after the relc pair so the DVE queue
        # serves the handover chain first.
        S_unit(h, 4)
        if h > 0:
            finish_head(h - 1)
        AV_unit(h, 0)
        if h == 0:
            v_mm(6)
            v_copy(6)
            v_mm(7)
            v_copy(7)
            phase_rel(1)

        # slot 4
        S_unit(h, 5)
        AV_unit(h, 1)
        if h == 0:
            ch = phase_relch(1)
            phase_relcw(1, ch)

        # slot 5
        S_unit(h, 6)
        AV_unit(h, 2)


        # slot 6
        S_unit(h, 7)
        AV_unit(h, 3)


        # slot 7
        AV_unit(h, 4)
        if have_next:
            S_unit(nh, 0)
        # qk for head h+2: emitted last so it has the lowest scheduler
        # priority — it fills PE idle gaps but yields to every S unit.
        if nh2 < HPC:
            for kt in range(6):
                qk_ktile(nh2, kt)

    # ---- epilogue ----
    # proj is restructured around the transp2 XBAR latency (~2us after the
    # last norms): the t0/t1 partial for every m-block is computed early and
    # spilled to SBUF (bf16); after transp2 lands, each m needs only an
    # identity-reinjection matmul (partial re-enters psum via I @ partial)
    # plus the t2 matmuls, copies, and the out DMA.
    def proj_pool(m):
        return [(ps_s, "ps"), (ps_qk, "pqk"), (ps_s, "ps"), (ps_av, "pav")][m % 4]

    def proj_t01(m):
        _mark(nc, f"proj{m}")
        pool, tag = proj_pool(m)
        pp = state[f"pp{m}"] = pool.tile([128, DIM], F32, tag=tag, name="pp")
        for t in range(2):
            for n0, nw in ((0, 512), (512, 256)):
                nc.tensor.matmul(pp[:, n0:n0 + nw],
                                 proj_lhsT[t][:, m * 128:(m + 1) * 128],
                                 wp_sb[:, t, n0:n0 + nw],
                                 start=(t == 0), stop=False)

    def proj_t2_out(m):
        _mark(nc, f"projo{m}")
        pp = state.pop(f"pp{m}")
        for n0, nw in ((0, 512), (512, 256)):
            nc.tensor.matmul(pp[:, n0:n0 + nw],
                             proj_lhsT[2][:, m * 128:(m + 1) * 128],
                             wp_sb[:, 2, n0:n0 + nw],
                             start=False, stop=True)
        osb = outp.tile([128, DIM], BF16, tag="osb", name="osb")
        nc.scalar.copy(osb[:, 0:384], pp[:, 0:384])
        nc.vector.tensor_copy(osb[:, 384:768], pp[:, 384:768])
        eng = nc.sync if m % 2 == 0 else nc.scalar
        eng.dma_start(out_d[m * 128:(m + 1) * 128, :], osb[:])

    for kb in (5, 6, 7):
        AV_unit(HPC - 1, kb)
    for m in range(4):
        proj_t01(m)
    finish_head(HPC - 1, last=True)
    for m in range(4):
        proj_t2_out(m)
        proj_t01(m + 4)
    for m in range(4, 8):
        proj_t2_out(m)


def _host_prep(x, qkv_w, qkv_b, proj_w, proj_b, rel_pos_h, rel_pos_w):
    bf = ml_dtypes.bfloat16
    idx_h = np.arange(H)[:, None] - np.arange(H)[None, :] + (H - 1)
    idx_w = np.arange(W)[:, None] - np.arange(W)[None, :] + (W - 1)
    Rh = rel_pos_h[idx_h]  # [qh, kh, c]
    Rw = rel_pos_w[idx_w]  # [qw, kw, c]
    rhT8 = np.ascontiguousarray((8.0 * Rh).transpose(2, 0, 1).reshape(HD, H * H)).astype(bf)
    rwT8 = np.ascontiguousarray((8.0 * Rw).transpose(2, 0, 1).reshape(HD, W * W)).astype(bf)
    kt = np.arange(N)
    ec = np.zeros((64, N), np.float32)
    ec[:32] = (np.arange(32)[:, None] == (kt // 32)[None, :])
    ec[32:] = (np.arange(32)[:, None] == (kt % 32)[None, :])
    ec = ec.astype(bf)

    in_maps = []
    for core in range(NCORES):
        b = core // 2
        h0 = (core % 2) * HPC
        xTc = np.ascontiguousarray(x[b].reshape(N, DIM).T).astype(bf)
        wqkc = np.zeros((DIM, HPC * 128), np.float32)
        wvc = np.zeros((DIM, HPC * 64), np.float32)
        wpc = np.zeros((HPC * HD, DIM), np.float32)
        qkbc = np.zeros((128, HPC), np.float32)
        for h in range(HPC):
            g = h0 + h
            wqkc[:, h * 128:h * 128 + 64] = qkv_w[g * HD:(g + 1) * HD].T
            wqkc[:, h * 128 + 64:h * 128 + 128] = qkv_w[DIM + g * HD:DIM + (g + 1) * HD].T
            wvc[:, h * 64:(h + 1) * 64] = qkv_w[2 * DIM + g * HD:2 * DIM + (g + 1) * HD].T
            wpc[h * HD:(h + 1) * HD, :] = proj_w[:, g * HD:(g + 1) * HD].T
            qkbc[0:64, h] = qkv_b[g * HD:(g + 1) * HD] * 0.125
            qkbc[64:128, h] = qkv_b[DIM + g * HD:DIM + (g + 1) * HD]
        in_maps.append({
            "xT": xTc, "wqk": wqkc.astype(bf), "wv": wvc.astype(bf),
            "wp": wpc.astype(bf), "rhT": rhT8, "rwT": rwT8, "ecomb": ec,
            "qkb": qkbc, "ident": np.eye(128, dtype=bf),
        })
    return in_maps


def kernel(x, qkv_w, qkv_b, proj_w, proj_b, rel_pos_h, rel_pos_w, _trace=False):
    x = np.asarray(x, np.float32)
    qkv_w = np.asarray(qkv_w, np.float32)
    qkv_b = np.asarray(qkv_b, np.float32)
    proj_w = np.asarray(proj_w, np.float32)
    proj_b = np.asarray(proj_b, np.float32)
    rel_pos_h = np.asarray(rel_pos_h, np.float32)
    rel_pos_w = np.asarray(rel_pos_w, np.float32)

    in_maps = _host_prep(x, qkv_w, qkv_b, proj_w, proj_b, rel_pos_h, rel_pos_w)
    if "nc" not in _cache:
        _cache["nc"] = build_program()
    nc = _cache["nc"]
    res = run_bass_kernel_spmd(nc, in_maps, core_ids=list(range(NCORES)),
                               trace=_trace)
    parts = [np.asarray(r["out_part"], np.float32) for r in res.results]
    # v-bias enters the output as a constant row: bv @ proj_w.T (attn rows sum
    # to one), folded here together with proj_b.
    bias_row = proj_b + qkv_b[2 * DIM:] @ proj_w.T
    out = np.zeros((B, N, DIM), np.float32)
    for b in range(B):
        out[b] = parts[2 * b] + parts[2 * b + 1] + bias_row
    if _trace:
        kernel.last_results = res
    return out.reshape(B, H, W, DIM)


# revision 44
# speedup vs baseline: 1.0085x; 1.0085x over previous
"""Trainium2 Bass kernel for decomposed-rel-pos attention (B=4, H=W=32, DIM=768, HEADS=12).

Sharding: 48 (batch, head) pairs -> 8 cores x 6 heads (core c: batch c//2,
heads (c%2)*6 .. +6). All matmul operands bf16 (fp32 PSUM accumulation),
bf16 output partials summed on host in fp32.

v2 schedule: engines execute in program order, so emission order IS the
schedule. PE is the binding engine (~67us of column-streaming); the program
is laid out so PE never head-of-line blocks:
 - per-head ladder of 8 slots; each slot emits the next S matmul pair first,
   then bounded filler (qk 3mm/slot over slots 0-3, AV units lag 3, the last
   three AV units of head h run in slots 0-2 of ladder h+1 so the tail burst
   never delays the next head's S).
 - the qk->extract->rel->rel-copy chain for head h+1 runs inside ladder h:
   extract-k on ACT (its slack engine), extract-q/rel copies on DVE,
   exp(h,4) on DVE via int16-Schraudolph bitcast (rel err ~2% on 1/8 of
   attn mass) to keep ACT <= PE.
 - prologue: qk(0) matmuls emitted per k-tile so they pipeline behind the
   per-tile x DMA; S(0,0) is split into k-contraction + bias-contraction
   halves so exp0 starts before the rel copies complete.
 - V projections (all 8 token-tiles) fill ladder-0/1 slots, each just ahead
   of its AV deadline.
 - epilogue: proj m-tiles on 4 rotating psum pools right after the last AV
   snapshot; output copies split ACT||DVE; out DMA alternates two queues.
"""
from contextlib import ExitStack

import numpy as np
import ml_dtypes

import concourse.bass as bass
import concourse.bacc as bacc
import concourse.mybir as mybir
import concourse.tile as tile
from concourse.bass_utils import run_bass_kernel_spmd

B, H, W, DIM, HEADS = 4, 32, 32, 768, 12
HD = DIM // HEADS  # 64
N = H * W  # 1024
HPC = HEADS // 2  # heads per core = 6
NCORES = 8
F32 = mybir.dt.float32
BF16 = mybir.dt.bfloat16
I16 = mybir.dt.int16
EXPF = mybir.ActivationFunctionType.Exp
IDENT = mybir.ActivationFunctionType.Identity
MUL = mybir.AluOpType.mult
ADD = mybir.AluOpType.add

# Schraudolph bf16 exp: i16 = round(x * 2^7/ln2 + (127*128 - 5.5)); bits are
# the bf16 value of ~exp(x) (max rel err 3.3%). Used for one of 8 exp tiles
# per head to offload ACT.
A16 = float(2 ** 7 / np.log(2))
B16 = float(127 * 128 - 5.5)

_cache = {}
PHASES = []  # (label, next_instruction_number) — filled during build for tracing


def _mark(nc, label):
    PHASES.append((label, int(nc.get_next_instruction_name()[2:])))


def build_program(reps=1):
    nc = bacc.Bacc("TRN2", target_bir_lowering=False, debug=False,
                   enable_asserts=False, num_devices=NCORES)
    xT = nc.dram_tensor("xT", [DIM, N], BF16, kind="ExternalInput")
    wqk = nc.dram_tensor("wqk", [DIM, HPC * 128], BF16, kind="ExternalInput")
    wv = nc.dram_tensor("wv", [DIM, HPC * 64], BF16, kind="ExternalInput")
    wp = nc.dram_tensor("wp", [HPC * HD, DIM], BF16, kind="ExternalInput")
    rhT = nc.dram_tensor("rhT", [HD, N], BF16, kind="ExternalInput")
    rwT = nc.dram_tensor("rwT", [HD, N], BF16, kind="ExternalInput")
    ecomb = nc.dram_tensor("ecomb", [64, N], BF16, kind="ExternalInput")
    qkb = nc.dram_tensor("qkb", [128, HPC], F32, kind="ExternalInput")
    out_d = nc.dram_tensor("out_part", [N, DIM], BF16, kind="ExternalOutput")

    with ExitStack() as ctx:
        tc = ctx.enter_context(tile.TileContext(nc))
        _body(nc, tc, ctx, xT, wqk, wv, wp, rhT, rwT, ecomb, qkb, out_d)
    nc.compile()
    return nc


def _body(nc, tc, ctx, xT, wqk, wv, wp, rhT, rwT, ecomb, qkb, out_d):
    persist = ctx.enter_context(tc.tile_pool(name="persist", bufs=1))
    small = ctx.enter_context(tc.tile_pool(name="small", bufs=3))
    outp = ctx.enter_context(tc.tile_pool(name="outp", bufs=8))
    ps_s = ctx.enter_context(tc.tile_pool(name="ps_s", bufs=2, space="PSUM"))
    ps_qk = ctx.enter_context(tc.tile_pool(name="ps_qk", bufs=1, space="PSUM"))
    ps_av = ctx.enter_context(tc.tile_pool(name="ps_av", bufs=1, space="PSUM"))

    # ---- static SBUF tiles ----
    xt_sb = persist.tile([128, 6, N], BF16, tag="xt", name="xt")
    wqk_sb = persist.tile([128, 6, HPC * 128], BF16, tag="wqk", name="wqk")
    wv_sb = persist.tile([128, 6, HPC * 64], BF16, tag="wv", name="wv")
    wp_sb = persist.tile([128, 3, DIM], BF16, tag="wp", name="wp")
    rhT_sb = persist.tile([HD, N], BF16, tag="rhT", name="rhT")
    rwT_sb = persist.tile([HD, N], BF16, tag="rwT", name="rwT")
    qkb_sb = persist.tile([128, HPC], F32, tag="qkb", name="qkb")
    lhsT = [persist.tile([128, N], BF16, tag=f"lhsT{p}", name=f"lhsT{p}") for p in range(2)]
    rhs_c = [persist.tile([128, N], BF16, tag=f"rhs{p}", name=f"rhs{p}") for p in range(2)]
    v_sb = [persist.tile([128, HPC * 65], BF16, tag=f"v{m}", name=f"v{m}") for m in range(8)]
    attnT = [persist.tile([128, N], BF16, tag=f"attnT{kb}", name=f"attnT{kb}") for kb in range(8)]
    attn_out = [persist.tile([128, N], BF16, tag=f"ao{j}", name=f"ao{j}") for j in range(3)]
    proj_lhsT = [persist.tile([128, N], BF16, tag=f"pl{j}", name=f"pl{j}") for j in range(3)]

    # ---- input DMA ----
    # The HWDGE descriptor generator is a single shared serial device
    # (~630ns per dma_start) and transfers serialize on the DMA bus, so:
    # few instructions, ordered by consumption deadline. x in 3 two-ktile
    # chunks so qk0 pipelines behind the bus; everything qk0/S(0,*) needs
    # (wqk head-0 cols, qkb, rel tables, ecomb) lands before wv/wqk-rest/wp.
    xt3 = xT.rearrange("(kt p) c -> p kt c", p=128)
    wqk3 = wqk.rearrange("(kt p) c -> p kt c", p=128)
    nc.sync.dma_start(xt_sb[:, 0:2, :], xt3[:, 0:2, :])
    nc.scalar.dma_start(wqk_sb[:, :, 0:128], wqk3[:, :, 0:128])
    nc.sync.dma_start(xt_sb[:, 2:4, :], xt3[:, 2:4, :])
    nc.scalar.dma_start(qkb_sb[:], qkb[:])
    nc.sync.dma_start(xt_sb[:, 4:6, :], xt3[:, 4:6, :])
    nc.scalar.dma_start(rhT_sb[:], rhT[:])
    nc.scalar.dma_start(rwT_sb[:], rwT[:])
    nc.scalar.dma_start(rhs_c[0][64:128, :], ecomb[:])
    nc.scalar.dma_start(rhs_c[1][64:128, :], ecomb[:])
    nc.sync.dma_start(wqk_sb[:, :, 128:256], wqk3[:, :, 128:256])
    nc.sync.dma_start(wv_sb[:], wv.rearrange("(kt p) c -> p kt c", p=128))
    nc.sync.dma_start(wqk_sb[:, :, 256:768], wqk3[:, :, 256:768])
    nc.scalar.dma_start(wp_sb[:], wp.rearrange("(t p) c -> p t c", p=128))

    # ones columns of V (row-sum trick), written once
    for m in range(8):
        v3 = v_sb[m][:].rearrange("p (h c) -> p h c", c=65)
        nc.gpsimd.memset(v3[:, :, 64], 1.0)

    # prime the ACT table load (~1.3us) off the critical path at t~0
    dummy = persist.tile([1, 8], F32, tag="dummy", name="dummy")
    nc.gpsimd.memset(dummy[0:1, 0:4], 0.0)
    nc.scalar.activation(dummy[0:1, 4:8], dummy[0:1, 0:4], EXPF)
    # PE warm-up source (zeros): dummy matmuls keep the PE p-state ramp going
    # through the x-DMA window so qk0 runs at full clock.
    warm_sb = persist.tile([128, 512], BF16, tag="warm", name="warm")
    nc.gpsimd.memset(warm_sb[:], 0.0)
    warm_ps = ps_av.tile([128, 512], F32, tag="pav", name="warm_ps")

    def warm(n):
        for _ in range(n):
            nc.tensor.matmul(warm_ps[:], warm_sb[:, 0:128], warm_sb[:],
                             start=True, stop=True)


    # ---- phase helpers ----
    state = {}

    def qk_ktile(h, kt):
        """qk projection for head h, one k-tile (2 matmuls)."""
        if kt == 0:
            _mark(nc, f"qk{h}")
            state["pq"] = ps_qk.tile([128, N], F32, tag="pqk", name="pqk")
        pq = state["pq"]
        for half in range(2):
            sl = slice(half * 512, half * 512 + 512)
            nc.tensor.matmul(pq[:, sl], wqk_sb[:, kt, h * 128:(h + 1) * 128],
                             xt_sb[:, kt, sl], start=(kt == 0), stop=(kt == 5))
        return pq

    def phase_extract(h):
        """Both extracts on DVE: the ACT queue must stay pure exps — any
        insert there stretches the S<->exp psum-rotation loop (the period).
        The chain has a full ladder of lead time now, so DVE-serial is fine."""
        _mark(nc, f"extract{h}")
        pq, p = state["pq"], h % 2
        nc.vector.tensor_scalar(lhsT[p][0:64, :], pq[0:64, :],
                                0.125, qkb_sb[0:64, h:h + 1], MUL, ADD)
        nc.vector.tensor_scalar(rhs_c[p][0:64, :], pq[64:128, :],
                                qkb_sb[64:128, h:h + 1], None, ADD)

    def phase_rel_h(h):
        _mark(nc, f"relh{h}")
        pq, p = state["pq"], h % 2
        prh = pq[0:32, :]
        for qh in range(32):
            sl = slice(qh * 32, qh * 32 + 32)
            nc.tensor.matmul(prh[:, sl], rhT_sb[:, sl], lhsT[p][0:64, sl],
                             start=True, stop=True)

    def phase_rel_w(h):
        _mark(nc, f"relw{h}")
        pq, p = state["pq"], h % 2
        prw = pq[32:64, :]
        qT3 = lhsT[p][0:64, :].rearrange("p (a b) -> p b a", b=32)
        for qw in range(32):
            sl = slice(qw * 32, qw * 32 + 32)
            nc.tensor.matmul(prw[:, sl], rwT_sb[:, sl], qT3[:, qw, :],
                             start=True, stop=True)

    def phase_rel(h):
        phase_rel_h(h)
        phase_rel_w(h)

    def phase_relch(h):
        _mark(nc, f"relch{h}")
        return nc.vector.tensor_copy(lhsT[h % 2][64:96, :], state["pq"][0:32, :])

    def phase_relcw(h, ch_inst=None):
        _mark(nc, f"relcw{h}")
        prw_v = state["pq"][32:64, :].rearrange("p (a b) -> p b a", b=32)
        cw = nc.vector.tensor_copy(lhsT[h % 2][96:128, :], prw_v[:, :, :])
        if ch_inst is not None:
            # Tile adds a false WW dep between the two rel copies (disjoint
            # partition ranges of the same tile); drop it so they run in
            # parallel on ACT/DVE — this pair is on the handover chain.
            cw.ins.try_remove_dependency(ch_inst.ins.name)
        return cw

    def v_mm(m):
        _mark(nc, f"vmm{m}")
        pv = ps_s.tile([128, HPC * 64], F32, tag="ps", name="pv")
        state[f"pv{m}"] = pv
        for kt in range(6):
            nc.tensor.matmul(pv[:], xt_sb[:, kt, m * 128:(m + 1) * 128],
                             wv_sb[:, kt, :], start=(kt == 0), stop=(kt == 5))

    def v_copy(m):
        _mark(nc, f"vcp{m}")
        pv = state.pop(f"pv{m}")
        dst = v_sb[m][:].rearrange("p (h c) -> p h c", c=65)[:, :, 0:64]
        nc.vector.tensor_copy(dst, pv[:].rearrange("p (h c) -> p h c", c=64))

    def S_unit(h, kb, dve_exp=False, defer_exp=False):
        _mark(nc, f"S{h}.{kb}")
        p = h % 2
        ps = ps_s.tile([128, N], F32, tag="ps", name="s_ps")
        for half in range(2):
            sl = slice(half * 512, half * 512 + 512)
            nc.tensor.matmul(ps[:, sl], rhs_c[p][:, kb * 128:(kb + 1) * 128],
                             lhsT[p][:, sl], start=True, stop=True)
        if defer_exp:
            return lambda: _exp(kb, ps, dve_exp)
        _exp(kb, ps, dve_exp)

    def _exp(kb, ps, dve_exp):
        if dve_exp:
            nc.vector.tensor_scalar(attnT[kb][:].bitcast(I16), ps[:],
                                    A16, B16, MUL, ADD)
        else:
            nc.scalar.activation(attnT[kb][:], ps[:], EXPF)

    # AV psum layout: qb 0-3 at cols qb*65 (bank 0), qb 4-7 at 512+(qb-4)*65
    # (bank 1); accumulation start/stop is bank-granular.
    def avcol(qb):
        return qb * 65 if qb < 4 else 512 + (qb - 4) * 65

    def AV_unit(h, kb):
        _mark(nc, f"AV{h}.{kb}")
        if kb == 0:
            state["pav"] = ps_av.tile([128, 512 + 4 * 65], F32, tag="pav", name="pav")
        pav = state["pav"]
        for qb in range(8):
            c = avcol(qb)
            nc.tensor.matmul(pav[:, c:c + 65],
                             attnT[kb][:, qb * 128:(qb + 1) * 128],
                             v_sb[kb][:, h * 65:(h + 1) * 65],
                             start=(kb == 0 and qb % 4 == 0),
                             stop=(kb == 7 and qb % 4 == 3))

    def transp(j, half=None):
        _mark(nc, f"transp{j}")
        pl4 = proj_lhsT[j][:].rearrange("p (m t) -> p m t", t=128)
        if half is None:
            nc.sync.dma_start_transpose(pl4, attn_out[j][:])
        else:
            q0, q1 = [(0, 2), (2, 4), (4, 8)][half]
            nc.sync.dma_start_transpose(pl4[:, q0:q1, :],
                                        attn_out[j][:, q0 * 128:q1 * 128])

    def finish_head(h, last=False):
        """Denominators, reciprocal, AV snapshot, normalization, transpose.
        Runs after AV(h,7); for h<5 this is inside ladder h+1 (slot 2+)."""
        _mark(nc, f"fin{h}")
        pav = state.pop("pav")
        denom = small.tile([128, 8], F32, tag="denom", name="denom")
        recip = small.tile([128, 8], F32, tag="recip", name="recip")
        pva = pav[:, 0:260].rearrange("p (a b) -> p a b", b=65)
        pvb = pav[:, 512:772].rearrange("p (a b) -> p a b", b=65)
        nc.vector.tensor_copy(denom[:, 0:4], pva[:, :, 64])
        nc.vector.tensor_copy(denom[:, 4:8], pvb[:, :, 64])
        nc.vector.reciprocal_approx_fast(out=recip[:], in_=denom[:])

        pavs = small.tile([128, 520], F32, tag="pavs", name="pavs")
        # snapshot releases the AV psum; the last head splits it ACT||DVE
        # (ACT is idle after the final exp).
        (nc.scalar.copy if last else nc.vector.tensor_copy)(
            pavs[:, 260:520], pav[:, 512:772])
        nc.vector.tensor_copy(pavs[:, 0:260], pav[:, 0:260])

        def dst_of(qb):
            return attn_out[h // 2][:, qb * 128 + (h % 2) * 64:
                                    qb * 128 + (h % 2) * 64 + 64]

        for qb in range(8):
            eng = nc.vector if qb % 2 == 0 else nc.gpsimd
            eng.tensor_scalar(dst_of(qb), pavs[:, qb * 65:qb * 65 + 64],
                              recip[:, qb:qb + 1], None, MUL)
            if last and qb == 1:
                transp(2, half=0)
            elif last and qb == 3:
                transp(2, half=1)
        if last:
            transp(2, half=2)
        elif h == 1:
            transp(0)
        elif h == 3:
            transp(1)

    # ---- prologue: head 0 ----
    # qk(0) per k-tile pipelines behind the x-chunk DMAs; warm-up matmuls
    # fill the DMA waits and keep the PE clock ramped.
    for kt in range(6):
        pq = qk_ktile(0, kt)
        if kt % 2 == 1:
            warm(4)
    _mark(nc, "extract0")
    # parallel extract: k rows on ACT (idle until first exp), q rows on DVE
    nc.scalar.activation(rhs_c[0][0:64, :], pq[64:128, :], IDENT,
                         bias=qkb_sb[64:128, 0:1])
    nc.vector.tensor_scalar(lhsT[0][0:64, :], pq[0:64, :],
                            0.125, qkb_sb[0:64, 0:1], MUL, ADD)
    phase_rel(0)
    # S(0,0) split into k-contraction half (ready after extracts) and bias
    # half (after rel copies) so exp0 starts ~1us earlier.
    _mark(nc, "S0.0")
    ps00 = ps_s.tile([128, N], F32, tag="ps", name="s_ps")
    for half in range(2):
        sl = slice(half * 512, half * 512 + 512)
        nc.tensor.matmul(ps00[:, sl], rhs_c[0][0:64, 0:128],
                         lhsT[0][0:64, sl], start=True, stop=False)
    _mark(nc, "relc0")
    ch0 = nc.scalar.copy(lhsT[0][64:96, :], pq[0:32, :])
    phase_relcw(0, ch0)
    _mark(nc, "S0.0b")
    for half in range(2):
        sl = slice(half * 512, half * 512 + 512)
        nc.tensor.matmul(ps00[:, sl], rhs_c[0][64:128, 0:128],
                         lhsT[0][64:128, sl], start=False, stop=True)
    nc.scalar.activation(attnT[0][:], ps00[:], EXPF)

    # ---- ladders ----
    # Slot plan for ladder h (steady state h>=1, nh=h+1 prepared):
    #   slot 0..2: S(h,1..3) + AV(h-1,5..7) + qk(nh) kt 2k,2k+1... (3mm/slot
    #              over slots 0-3) ; finish_head(h-1) after AV(h-1,7)
    #   slot 3:    S(h,4) + AV(h,0) + qk rest + extract(nh)
    #   slot 4:    S(h,5) + AV(h,1)   [S(h,4) exp on DVE]
    #   slot 5:    S(h,6) + AV(h,2) + rel(nh)
    #   slot 6:    S(h,7) + AV(h,3) + relch/relcw(nh)
    #   slot 7:    AV(h,4) + S(nh,0)
    # Ladder 0 additionally carries the 8 V projections (deadline: AV(0,m)).
    # Steady-state chain pipelining: qk for head h+2 is emitted in slots 5-7
    # of ladder h (the qk psum frees once head h+1's rel copies read it at
    # slot 2), so when ladder h+1 starts, extract(h+2) can run immediately at
    # slot 0, rel at slot 1, rel copies at slot 2 — the handover chain
    # completes ~4 slots before S(h+2, 0) needs it.
    for h in range(HPC):
        nh, nh2 = h + 1, h + 2
        have_next = nh < HPC

        # slot 0
        if h == 0:
            v_mm(0)
            v_copy(0)
            v_mm(1)
            v_copy(1)
            for kt in range(6):
                qk_ktile(1, kt)  # one-time burst; ladder-0 slots are light
        else:
            S_unit(h, 1)
            AV_unit(h - 1, 5)
            if have_next:
                phase_extract(nh)

        # slot 1
        if h == 0:
            S_unit(0, 1)
            v_mm(2)
            v_copy(2)
            v_mm(3)
            v_copy(3)
            phase_extract(1)
        else:
            S_unit(h, 2)
            AV_unit(h - 1, 6)
            if have_next:
                phase_rel_h(nh)

        # slot 2
        if h == 0:
            S_unit(0, 2)
            v_mm(4)
            v_copy(4)
            v_mm(5)
            v_copy(5)
            S_unit(0, 3)
        else:
            S_unit(h, 3)
            AV_unit(h - 1, 7)
            if have_next:
                phase_rel_w(nh)
                ch = phase_relch(nh)
                phase_relcw(nh, ch)

        # slot 3  (qk(nh2) may only start after relc(nh) is emitted: the qk
        # psum WAR dep must point at already-emitted readers — so in ladder 0,
        # where relc(1) lands at slot 4, qk(2) waits until slot 5).
        # finish_head(h-1) is emitted after the relc pair so the DVE queue
        # serves the handover chain first.
        S_unit(h, 4)
        if h > 0:
            finish_head(h - 1)
        AV_unit(h, 0)
        if h == 0:
            v_mm(6)
            v_copy(6)
            v_mm(7)
            v_copy(7)
            phase_rel(1)

        # slot 4
        S_unit(h, 5)
        AV_unit(h, 1)
        if h == 0:
            ch = phase_relch(1)
            phase_relcw(1, ch)

        # slot 5
        S_unit(h, 6)
        AV_unit(h, 2)


        # slot 6
        S_unit(h, 7)
        AV_unit(h, 3)


        # slot 7
        AV_unit(h, 4)
        if have_next:
            S_unit(nh, 0)
        # qk for head h+2: emitted last so it has the lowest scheduler
        # priority — it fills PE idle gaps but yields to every S unit.
        if nh2 < HPC:
            for kt in range(6):
                qk_ktile(nh2, kt)

    # ---- epilogue ----
    # proj is restructured around the transp2 XBAR latency (~2us after the
    # last norms): the t0/t1 partial for every m-block is computed early and
    # spilled to SBUF (bf16); after transp2 lands, each m needs only an
    # identity-reinjection matmul (partial re-enters psum via I @ partial)
    # plus the t2 matmuls, copies, and the out DMA.
    def proj_pool(m):
        return [(ps_s, "ps"), (ps_qk, "pqk"), (ps_s, "ps"), (ps_av, "pav")][m % 4]

    def proj_t01(m):
        _mark(nc, f"proj{m}")
        pool, tag = proj_pool(m)
        pp = state[f"pp{m}"] = pool.tile([128, DIM], F32, tag=tag, name="pp")
        for t in range(2):
            for n0, nw in ((0, 512), (512, 256)):
                nc.tensor.matmul(pp[:, n0:n0 + nw],
                                 proj_lhsT[t][:, m * 128:(m + 1) * 128],
                                 wp_sb[:, t, n0:n0 + nw],
                                 start=(t == 0), stop=False)

    def proj_t2_out(m):
        _mark(nc, f"projo{m}")
        pp = state.pop(f"pp{m}")
        for n0, nw in ((0, 512), (512, 256)):
            nc.tensor.matmul(pp[:, n0:n0 + nw],
                             proj_lhsT[2][:, m * 128:(m + 1) * 128],
                             wp_sb[:, 2, n0:n0 + nw],
                             start=False, stop=True)
        osb = outp.tile([128, DIM], BF16, tag="osb", name="osb")
        nc.scalar.copy(osb[:, 0:384], pp[:, 0:384])
        nc.vector.tensor_copy(osb[:, 384:768], pp[:, 384:768])
        eng = nc.sync if m % 2 == 0 else nc.scalar
        eng.dma_start(out_d[m * 128:(m + 1) * 128, :], osb[:])

    for kb in (5, 6, 7):
        AV_unit(HPC - 1, kb)
    for m in range(4):
        proj_t01(m)
    finish_head(HPC - 1, last=True)
    for m in range(4):
        proj_t2_out(m)
        proj_t01(m + 4)
    for m in range(4, 8):
        proj_t2_out(m)


def _host_prep(x, qkv_w, qkv_b, proj_w, proj_b, rel_pos_h, rel_pos_w):
    bf = ml_dtypes.bfloat16
    idx_h = np.arange(H)[:, None] - np.arange(H)[None, :] + (H - 1)
    idx_w = np.arange(W)[:, None] - np.arange(W)[None, :] + (W - 1)
    Rh = rel_pos_h[idx_h]  # [qh, kh, c]
    Rw = rel_pos_w[idx_w]  # [qw, kw, c]
    rhT8 = np.ascontiguousarray((8.0 * Rh).transpose(2, 0, 1).reshape(HD, H * H)).astype(bf)
    rwT8 = np.ascontiguousarray((8.0 * Rw).transpose(2, 0, 1).reshape(HD, W * W)).astype(bf)
    kt = np.arange(N)
    ec = np.zeros((64, N), np.float32)
    ec[:32] = (np.arange(32)[:, None] == (kt // 32)[None, :])
    ec[32:] = (np.arange(32)[:, None] == (kt % 32)[None, :])
    ec = ec.astype(bf)

    in_maps = []
    for core in range(NCORES):
        b = core // 2
        h0 = (core % 2) * HPC
        xTc = np.ascontiguousarray(x[b].reshape(N, DIM).T).astype(bf)
        wqkc = np.zeros((DIM, HPC * 128), np.float32)
        wvc = np.zeros((DIM, HPC * 64), np.float32)
        wpc = np.zeros((HPC * HD, DIM), np.float32)
        qkbc = np.zeros((128, HPC), np.float32)
        for h in range(HPC):
            g = h0 + h
            wqkc[:, h * 128:h * 128 + 64] = qkv_w[g * HD:(g + 1) * HD].T
            wqkc[:, h * 128 + 64:h * 128 + 128] = qkv_w[DIM + g * HD:DIM + (g + 1) * HD].T
            wvc[:, h * 64:(h + 1) * 64] = qkv_w[2 * DIM + g * HD:2 * DIM + (g + 1) * HD].T
            wpc[h * HD:(h + 1) * HD, :] = proj_w[:, g * HD:(g + 1) * HD].T
            qkbc[0:64, h] = qkv_b[g * HD:(g + 1) * HD] * 0.125
            qkbc[64:128, h] = qkv_b[DIM + g * HD:DIM + (g + 1) * HD]
        in_maps.append({
            "xT": xTc, "wqk": wqkc.astype(bf), "wv": wvc.astype(bf),
            "wp": wpc.astype(bf), "rhT": rhT8, "rwT": rwT8, "ecomb": ec,
            "qkb": qkbc,
        })
    return in_maps


def kernel(x, qkv_w, qkv_b, proj_w, proj_b, rel_pos_h, rel_pos_w, _trace=False):
    x = np.asarray(x, np.float32)
    qkv_w = np.asarray(qkv_w, np.float32)
    qkv_b = np.asarray(qkv_b, np.float32)
    proj_w = np.asarray(proj_w, np.float32)
    proj_b = np.asarray(proj_b, np.float32)
    rel_pos_h = np.asarray(rel_pos_h, np.float32)
    rel_pos_w = np.asarray(rel_pos_w, np.float32)

    in_maps = _host_prep(x, qkv_w, qkv_b, proj_w, proj_b, rel_pos_h, rel_pos_w)
    if "nc" not in _cache:
        _cache["nc"] = build_program()
    nc = _cache["nc"]
    res = run_bass_kernel_spmd(nc, in_maps, core_ids=list(range(NCORES)),
                               trace=_trace)
    parts = [np.asarray(r["out_part"], np.float32) for r in res.results]
    # v-bias enters the output as a constant row: bv @ proj_w.T (attn rows sum
    # to one), folded here together with proj_b.
    bias_row = proj_b + qkv_b[2 * DIM:] @ proj_w.T
    out = np.zeros((B, N, DIM), np.float32)
    for b in range(B):
        out[b] = parts[2 * b] + parts[2 * b + 1] + bias_row
    if _trace:
        kernel.last_results = res
    return out.reshape(B, H, W, DIM)


# revision 45
# speedup vs baseline: 1.0092x; 1.0007x over previous
"""Trainium2 Bass kernel for decomposed-rel-pos attention (B=4, H=W=32, DIM=768, HEADS=12).

Sharding: 48 (batch, head) pairs -> 8 cores x 6 heads (core c: batch c//2,
heads (c%2)*6 .. +6). All matmul operands bf16 (fp32 PSUM accumulation),
bf16 output partials summed on host in fp32.

v2 schedule: engines execute in program order, so emission order IS the
schedule. PE is the binding engine (~67us of column-streaming); the program
is laid out so PE never head-of-line blocks:
 - per-head ladder of 8 slots; each slot emits the next S matmul pair first,
   then bounded filler (qk 3mm/slot over slots 0-3, AV units lag 3, the last
   three AV units of head h run in slots 0-2 of ladder h+1 so the tail burst
   never delays the next head's S).
 - the qk->extract->rel->rel-copy chain for head h+1 runs inside ladder h:
   extract-k on ACT (its slack engine), extract-q/rel copies on DVE,
   exp(h,4) on DVE via int16-Schraudolph bitcast (rel err ~2% on 1/8 of
   attn mass) to keep ACT <= PE.
 - prologue: qk(0) matmuls emitted per k-tile so they pipeline behind the
   per-tile x DMA; S(0,0) is split into k-contraction + bias-contraction
   halves so exp0 starts before the rel copies complete.
 - V projections (all 8 token-tiles) fill ladder-0/1 slots, each just ahead
   of its AV deadline.
 - epilogue: proj m-tiles on 4 rotating psum pools right after the last AV
   snapshot; output copies split ACT||DVE; out DMA alternates two queues.
"""
from contextlib import ExitStack

import numpy as np
import ml_dtypes

import concourse.bass as bass
import concourse.bacc as bacc
import concourse.mybir as mybir
import concourse.tile as tile
from concourse.bass_utils import run_bass_kernel_spmd

B, H, W, DIM, HEADS = 4, 32, 32, 768, 12
HD = DIM // HEADS  # 64
N = H * W  # 1024
HPC = HEADS // 2  # heads per core = 6
NCORES = 8
F32 = mybir.dt.float32
BF16 = mybir.dt.bfloat16
I16 = mybir.dt.int16
EXPF = mybir.ActivationFunctionType.Exp
IDENT = mybir.ActivationFunctionType.Identity
MUL = mybir.AluOpType.mult
ADD = mybir.AluOpType.add

# Schraudolph bf16 exp: i16 = round(x * 2^7/ln2 + (127*128 - 5.5)); bits are
# the bf16 value of ~exp(x) (max rel err 3.3%). Used for one of 8 exp tiles
# per head to offload ACT.
A16 = float(2 ** 7 / np.log(2))
B16 = float(127 * 128 - 5.5)

_cache = {}
PHASES = []  # (label, next_instruction_number) — filled during build for tracing


def _mark(nc, label):
    PHASES.append((label, int(nc.get_next_instruction_name()[2:])))


def build_program(reps=1):
    nc = bacc.Bacc("TRN2", target_bir_lowering=False, debug=False,
                   enable_asserts=False, num_devices=NCORES)
    xT = nc.dram_tensor("xT", [DIM, N], BF16, kind="ExternalInput")
    wqk = nc.dram_tensor("wqk", [DIM, HPC * 128], BF16, kind="ExternalInput")
    wv = nc.dram_tensor("wv", [DIM, HPC * 64], BF16, kind="ExternalInput")
    wp = nc.dram_tensor("wp", [HPC * HD, DIM], BF16, kind="ExternalInput")
    rhT = nc.dram_tensor("rhT", [HD, N], BF16, kind="ExternalInput")
    rwT = nc.dram_tensor("rwT", [HD, N], BF16, kind="ExternalInput")
    ecomb = nc.dram_tensor("ecomb", [64, N], BF16, kind="ExternalInput")
    qkb = nc.dram_tensor("qkb", [128, HPC], F32, kind="ExternalInput")
    out_d = nc.dram_tensor("out_part", [N, DIM], BF16, kind="ExternalOutput")

    with ExitStack() as ctx:
        tc = ctx.enter_context(tile.TileContext(nc))
        _body(nc, tc, ctx, xT, wqk, wv, wp, rhT, rwT, ecomb, qkb, out_d)
    nc.compile()
    return nc


def _body(nc, tc, ctx, xT, wqk, wv, wp, rhT, rwT, ecomb, qkb, out_d):
    persist = ctx.enter_context(tc.tile_pool(name="persist", bufs=1))
    small = ctx.enter_context(tc.tile_pool(name="small", bufs=3))
    outp = ctx.enter_context(tc.tile_pool(name="outp", bufs=8))
    ps_s = ctx.enter_context(tc.tile_pool(name="ps_s", bufs=2, space="PSUM"))
    ps_qk = ctx.enter_context(tc.tile_pool(name="ps_qk", bufs=1, space="PSUM"))
    ps_av = ctx.enter_context(tc.tile_pool(name="ps_av", bufs=1, space="PSUM"))

    # ---- static SBUF tiles ----
    xt_sb = persist.tile([128, 6, N], BF16, tag="xt", name="xt")
    wqk_sb = persist.tile([128, 6, HPC * 128], BF16, tag="wqk", name="wqk")
    wv_sb = persist.tile([128, 6, HPC * 64], BF16, tag="wv", name="wv")
    wp_sb = persist.tile([128, 3, DIM], BF16, tag="wp", name="wp")
    rhT_sb = persist.tile([HD, N], BF16, tag="rhT", name="rhT")
    rwT_sb = persist.tile([HD, N], BF16, tag="rwT", name="rwT")
    qkb_sb = persist.tile([128, HPC], F32, tag="qkb", name="qkb")
    lhsT = [persist.tile([128, N], BF16, tag=f"lhsT{p}", name=f"lhsT{p}") for p in range(2)]
    rhs_c = [persist.tile([128, N], BF16, tag=f"rhs{p}", name=f"rhs{p}") for p in range(2)]
    v_sb = [persist.tile([128, HPC * 65], BF16, tag=f"v{m}", name=f"v{m}") for m in range(8)]
    attnT = [persist.tile([128, N], BF16, tag=f"attnT{kb}", name=f"attnT{kb}") for kb in range(8)]
    attn_out = [persist.tile([128, N], BF16, tag=f"ao{j}", name=f"ao{j}") for j in range(3)]
    proj_lhsT = [persist.tile([128, N], BF16, tag=f"pl{j}", name=f"pl{j}") for j in range(3)]

    # ---- input DMA ----
    # The HWDGE descriptor generator is a single shared serial device
    # (~630ns per dma_start) and transfers serialize on the DMA bus, so:
    # few instructions, ordered by consumption deadline. x in 3 two-ktile
    # chunks so qk0 pipelines behind the bus; everything qk0/S(0,*) needs
    # (wqk head-0 cols, qkb, rel tables, ecomb) lands before wv/wqk-rest/wp.
    xt3 = xT.rearrange("(kt p) c -> p kt c", p=128)
    wqk3 = wqk.rearrange("(kt p) c -> p kt c", p=128)
    nc.sync.dma_start(xt_sb[:, 0:2, :], xt3[:, 0:2, :])
    nc.scalar.dma_start(wqk_sb[:, :, 0:128], wqk3[:, :, 0:128])
    nc.sync.dma_start(xt_sb[:, 2:4, :], xt3[:, 2:4, :])
    nc.scalar.dma_start(qkb_sb[:], qkb[:])
    nc.sync.dma_start(xt_sb[:, 4:6, :], xt3[:, 4:6, :])
    nc.scalar.dma_start(rhT_sb[:], rhT[:])
    nc.scalar.dma_start(rwT_sb[:], rwT[:])
    nc.scalar.dma_start(rhs_c[0][64:128, :], ecomb[:])
    nc.scalar.dma_start(rhs_c[1][64:128, :], ecomb[:])
    nc.sync.dma_start(wqk_sb[:, :, 128:256], wqk3[:, :, 128:256])
    nc.sync.dma_start(wv_sb[:], wv.rearrange("(kt p) c -> p kt c", p=128))
    nc.sync.dma_start(wqk_sb[:, :, 256:768], wqk3[:, :, 256:768])
    nc.scalar.dma_start(wp_sb[:], wp.rearrange("(t p) c -> p t c", p=128))

    # ones columns of V (row-sum trick), written once
    for m in range(8):
        v3 = v_sb[m][:].rearrange("p (h c) -> p h c", c=65)
        nc.gpsimd.memset(v3[:, :, 64], 1.0)

    # prime the ACT table load (~1.3us) off the critical path at t~0
    dummy = persist.tile([1, 8], F32, tag="dummy", name="dummy")
    nc.gpsimd.memset(dummy[0:1, 0:4], 0.0)
    nc.scalar.activation(dummy[0:1, 4:8], dummy[0:1, 0:4], EXPF)
    # PE warm-up source (zeros): dummy matmuls keep the PE p-state ramp going
    # through the x-DMA window so qk0 runs at full clock.
    warm_sb = persist.tile([128, 512], BF16, tag="warm", name="warm")
    nc.gpsimd.memset(warm_sb[:], 0.0)
    warm_ps = ps_av.tile([128, 512], F32, tag="pav", name="warm_ps")

    def warm(n):
        for _ in range(n):
            nc.tensor.matmul(warm_ps[:], warm_sb[:, 0:128], warm_sb[:],
                             start=True, stop=True)


    # ---- phase helpers ----
    state = {}

    def qk_ktile(h, kt):
        """qk projection for head h, one k-tile (2 matmuls)."""
        if kt == 0:
            _mark(nc, f"qk{h}")
            state["pq"] = ps_qk.tile([128, N], F32, tag="pqk", name="pqk")
        pq = state["pq"]
        for half in range(2):
            sl = slice(half * 512, half * 512 + 512)
            nc.tensor.matmul(pq[:, sl], wqk_sb[:, kt, h * 128:(h + 1) * 128],
                             xt_sb[:, kt, sl], start=(kt == 0), stop=(kt == 5))
        return pq

    def phase_extract(h):
        """Both extracts on DVE: the ACT queue must stay pure exps — any
        insert there stretches the S<->exp psum-rotation loop (the period).
        The chain has a full ladder of lead time now, so DVE-serial is fine."""
        _mark(nc, f"extract{h}")
        pq, p = state["pq"], h % 2
        nc.vector.tensor_scalar(lhsT[p][0:64, :], pq[0:64, :],
                                0.125, qkb_sb[0:64, h:h + 1], MUL, ADD)
        nc.vector.tensor_scalar(rhs_c[p][0:64, :], pq[64:128, :],
                                qkb_sb[64:128, h:h + 1], None, ADD)

    def phase_rel_h(h):
        _mark(nc, f"relh{h}")
        pq, p = state["pq"], h % 2
        prh = pq[0:32, :]
        for qh in range(32):
            sl = slice(qh * 32, qh * 32 + 32)
            nc.tensor.matmul(prh[:, sl], rhT_sb[:, sl], lhsT[p][0:64, sl],
                             start=True, stop=True)

    def phase_rel_w(h):
        _mark(nc, f"relw{h}")
        pq, p = state["pq"], h % 2
        prw = pq[32:64, :]
        qT3 = lhsT[p][0:64, :].rearrange("p (a b) -> p b a", b=32)
        for qw in range(32):
            sl = slice(qw * 32, qw * 32 + 32)
            nc.tensor.matmul(prw[:, sl], rwT_sb[:, sl], qT3[:, qw, :],
                             start=True, stop=True)

    def phase_rel(h):
        phase_rel_h(h)
        phase_rel_w(h)

    def phase_relch(h):
        _mark(nc, f"relch{h}")
        return nc.vector.tensor_copy(lhsT[h % 2][64:96, :], state["pq"][0:32, :])

    def phase_relcw(h, ch_inst=None):
        _mark(nc, f"relcw{h}")
        prw_v = state["pq"][32:64, :].rearrange("p (a b) -> p b a", b=32)
        cw = nc.vector.tensor_copy(lhsT[h % 2][96:128, :], prw_v[:, :, :])
        if ch_inst is not None:
            # Tile adds a false WW dep between the two rel copies (disjoint
            # partition ranges of the same tile); drop it so they run in
            # parallel on ACT/DVE — this pair is on the handover chain.
            cw.ins.try_remove_dependency(ch_inst.ins.name)
        return cw

    def v_mm(m):
        _mark(nc, f"vmm{m}")
        pv = ps_s.tile([128, HPC * 64], F32, tag="ps", name="pv")
        state[f"pv{m}"] = pv
        for kt in range(6):
            nc.tensor.matmul(pv[:], xt_sb[:, kt, m * 128:(m + 1) * 128],
                             wv_sb[:, kt, :], start=(kt == 0), stop=(kt == 5))

    def v_copy(m):
        _mark(nc, f"vcp{m}")
        pv = state.pop(f"pv{m}")
        dst = v_sb[m][:].rearrange("p (h c) -> p h c", c=65)[:, :, 0:64]
        nc.vector.tensor_copy(dst, pv[:].rearrange("p (h c) -> p h c", c=64))

    def S_unit(h, kb, dve_exp=False, defer_exp=False):
        _mark(nc, f"S{h}.{kb}")
        p = h % 2
        ps = ps_s.tile([128, N], F32, tag="ps", name="s_ps")
        for half in range(2):
            sl = slice(half * 512, half * 512 + 512)
            nc.tensor.matmul(ps[:, sl], rhs_c[p][:, kb * 128:(kb + 1) * 128],
                             lhsT[p][:, sl], start=True, stop=True)
        if defer_exp:
            return lambda: _exp(kb, ps, dve_exp)
        _exp(kb, ps, dve_exp)

    def _exp(kb, ps, dve_exp):
        if dve_exp:
            nc.vector.tensor_scalar(attnT[kb][:].bitcast(I16), ps[:],
                                    A16, B16, MUL, ADD)
        else:
            nc.scalar.activation(attnT[kb][:], ps[:], EXPF)

    # AV psum layout: qb 0-3 at cols qb*65 (bank 0), qb 4-7 at 512+(qb-4)*65
    # (bank 1); accumulation start/stop is bank-granular.
    def avcol(qb):
        return qb * 65 if qb < 4 else 512 + (qb - 4) * 65

    def AV_unit(h, kb):
        _mark(nc, f"AV{h}.{kb}")
        if kb == 0:
            state["pav"] = ps_av.tile([128, 512 + 4 * 65], F32, tag="pav", name="pav")
        pav = state["pav"]
        for qb in range(8):
            c = avcol(qb)
            nc.tensor.matmul(pav[:, c:c + 65],
                             attnT[kb][:, qb * 128:(qb + 1) * 128],
                             v_sb[kb][:, h * 65:(h + 1) * 65],
                             start=(kb == 0 and qb % 4 == 0),
                             stop=(kb == 7 and qb % 4 == 3))

    def transp(j, half=None):
        _mark(nc, f"transp{j}")
        pl4 = proj_lhsT[j][:].rearrange("p (m t) -> p m t", t=128)
        if half is None:
            nc.sync.dma_start_transpose(pl4, attn_out[j][:])
        else:
            q0, q1 = [(0, 2), (2, 4), (4, 8)][half]
            nc.sync.dma_start_transpose(pl4[:, q0:q1, :],
                                        attn_out[j][:, q0 * 128:q1 * 128])

    def finish_head(h, last=False):
        """Denominators, reciprocal, AV snapshot, normalization, transpose.
        Runs after AV(h,7); for h<5 this is inside ladder h+1 (slot 2+)."""
        _mark(nc, f"fin{h}")
        pav = state.pop("pav")
        denom = small.tile([128, 8], F32, tag="denom", name="denom")
        recip = small.tile([128, 8], F32, tag="recip", name="recip")
        pva = pav[:, 0:260].rearrange("p (a b) -> p a b", b=65)
        pvb = pav[:, 512:772].rearrange("p (a b) -> p a b", b=65)
        nc.vector.tensor_copy(denom[:, 0:4], pva[:, :, 64])
        nc.vector.tensor_copy(denom[:, 4:8], pvb[:, :, 64])
        nc.vector.reciprocal_approx_fast(out=recip[:], in_=denom[:])

        pavs = small.tile([128, 520], F32, tag="pavs", name="pavs")
        # snapshot releases the AV psum; the last head splits it ACT||DVE
        # (ACT is idle after the final exp).
        (nc.scalar.copy if last else nc.vector.tensor_copy)(
            pavs[:, 260:520], pav[:, 512:772])
        nc.vector.tensor_copy(pavs[:, 0:260], pav[:, 0:260])

        def dst_of(qb):
            return attn_out[h // 2][:, qb * 128 + (h % 2) * 64:
                                    qb * 128 + (h % 2) * 64 + 64]

        for qb in range(8):
            eng = nc.vector if qb % 2 == 0 else nc.gpsimd
            eng.tensor_scalar(dst_of(qb), pavs[:, qb * 65:qb * 65 + 64],
                              recip[:, qb:qb + 1], None, MUL)
            if last and qb == 1:
                transp(2, half=0)
            elif last and qb == 3:
                transp(2, half=1)
        if last:
            transp(2, half=2)
        elif h == 1:
            transp(0)
        elif h == 3:
            transp(1)

    # ---- prologue: head 0 ----
    # qk(0) per k-tile pipelines behind the x-chunk DMAs; warm-up matmuls
    # fill the DMA waits and keep the PE clock ramped.
    for kt in range(6):
        pq = qk_ktile(0, kt)
        if kt in (1, 3):
            warm(4)
    _mark(nc, "extract0")
    # parallel extract: q rows on DVE, k rows on ACT (idle until first exp)
    nc.vector.tensor_scalar(lhsT[0][0:64, :], pq[0:64, :],
                            0.125, qkb_sb[0:64, 0:1], MUL, ADD)
    nc.scalar.activation(rhs_c[0][0:64, :], pq[64:128, :], IDENT,
                         bias=qkb_sb[64:128, 0:1])
    phase_rel(0)
    # S(0,0) split into k-contraction half (ready after extracts) and bias
    # half (after rel copies) so exp0 starts ~1us earlier.
    _mark(nc, "S0.0")
    ps00 = ps_s.tile([128, N], F32, tag="ps", name="s_ps")
    for half in range(2):
        sl = slice(half * 512, half * 512 + 512)
        nc.tensor.matmul(ps00[:, sl], rhs_c[0][0:64, 0:128],
                         lhsT[0][0:64, sl], start=True, stop=False)
    _mark(nc, "relc0")
    ch0 = nc.scalar.copy(lhsT[0][64:96, :], pq[0:32, :])
    phase_relcw(0, ch0)
    _mark(nc, "S0.0b")
    for half in range(2):
        sl = slice(half * 512, half * 512 + 512)
        nc.tensor.matmul(ps00[:, sl], rhs_c[0][64:128, 0:128],
                         lhsT[0][64:128, sl], start=False, stop=True)
    nc.scalar.activation(attnT[0][:], ps00[:], EXPF)

    # ---- ladders ----
    # Slot plan for ladder h (steady state h>=1, nh=h+1 prepared):
    #   slot 0..2: S(h,1..3) + AV(h-1,5..7) + qk(nh) kt 2k,2k+1... (3mm/slot
    #              over slots 0-3) ; finish_head(h-1) after AV(h-1,7)
    #   slot 3:    S(h,4) + AV(h,0) + qk rest + extract(nh)
    #   slot 4:    S(h,5) + AV(h,1)   [S(h,4) exp on DVE]
    #   slot 5:    S(h,6) + AV(h,2) + rel(nh)
    #   slot 6:    S(h,7) + AV(h,3) + relch/relcw(nh)
    #   slot 7:    AV(h,4) + S(nh,0)
    # Ladder 0 additionally carries the 8 V projections (deadline: AV(0,m)).
    # Steady-state chain pipelining: qk for head h+2 is emitted in slots 5-7
    # of ladder h (the qk psum frees once head h+1's rel copies read it at
    # slot 2), so when ladder h+1 starts, extract(h+2) can run immediately at
    # slot 0, rel at slot 1, rel copies at slot 2 — the handover chain
    # completes ~4 slots before S(h+2, 0) needs it.
    for h in range(HPC):
        nh, nh2 = h + 1, h + 2
        have_next = nh < HPC

        # slot 0
        if h == 0:
            v_mm(0)
            v_copy(0)
            v_mm(1)
            v_copy(1)
            for kt in range(6):
                qk_ktile(1, kt)  # one-time burst; ladder-0 slots are light
        else:
            S_unit(h, 1)
            AV_unit(h - 1, 5)
            if have_next:
                phase_extract(nh)

        # slot 1
        if h == 0:
            S_unit(0, 1)
            v_mm(2)
            v_copy(2)
            v_mm(3)
            v_copy(3)
            phase_extract(1)
        else:
            S_unit(h, 2)
            AV_unit(h - 1, 6)
            if have_next:
                phase_rel_h(nh)

        # slot 2
        if h == 0:
            S_unit(0, 2)
            v_mm(4)
            v_copy(4)
            v_mm(5)
            v_copy(5)
            S_unit(0, 3)
        else:
            S_unit(h, 3)
            AV_unit(h - 1, 7)
            if have_next:
                phase_rel_w(nh)
                ch = phase_relch(nh)
                phase_relcw(nh, ch)

        # slot 3  (qk(nh2) may only start after relc(nh) is emitted: the qk
        # psum WAR dep must point at already-emitted readers — so in ladder 0,
        # where relc(1) lands at slot 4, qk(2) waits until slot 5).
        # finish_head(h-1) is emitted after the relc pair so the DVE queue
        # serves the handover chain first.
        S_unit(h, 4)
        if h > 0:
            finish_head(h - 1)
        AV_unit(h, 0)
        if h == 0:
            v_mm(6)
            v_copy(6)
            v_mm(7)
            v_copy(7)
            phase_rel(1)

        # slot 4
        S_unit(h, 5)
        AV_unit(h, 1)
        if h == 0:
            ch = phase_relch(1)
            phase_relcw(1, ch)

        # slot 5
        S_unit(h, 6)
        AV_unit(h, 2)


        # slot 6
        S_unit(h, 7)
        AV_unit(h, 3)


        # slot 7
        AV_unit(h, 4)
        if have_next:
            S_unit(nh, 0)
        # qk for head h+2: emitted last so it has the lowest scheduler
        # priority — it fills PE idle gaps but yields to every S unit.
        if nh2 < HPC:
            for kt in range(6):
                qk_ktile(nh2, kt)

    # ---- epilogue ----
    # proj is restructured around the transp2 XBAR latency (~2us after the
    # last norms): the t0/t1 partial for every m-block is computed early and
    # spilled to SBUF (bf16); after transp2 lands, each m needs only an
    # identity-reinjection matmul (partial re-enters psum via I @ partial)
    # plus the t2 matmuls, copies, and the out DMA.
    def proj_pool(m):
        return [(ps_s, "ps"), (ps_qk, "pqk"), (ps_s, "ps"), (ps_av, "pav")][m % 4]

    def proj_t01(m):
        _mark(nc, f"proj{m}")
        pool, tag = proj_pool(m)
        pp = state[f"pp{m}"] = pool.tile([128, DIM], F32, tag=tag, name="pp")
        for t in range(2):
            for n0, nw in ((0, 512), (512, 256)):
                nc.tensor.matmul(pp[:, n0:n0 + nw],
                                 proj_lhsT[t][:, m * 128:(m + 1) * 128],
                                 wp_sb[:, t, n0:n0 + nw],
                                 start=(t == 0), stop=False)

    def proj_t2_out(m):
        _mark(nc, f"projo{m}")
        pp = state.pop(f"pp{m}")
        for n0, nw in ((0, 512), (512, 256)):
            nc.tensor.matmul(pp[:, n0:n0 + nw],
                             proj_lhsT[2][:, m * 128:(m + 1) * 128],
                             wp_sb[:, 2, n0:n0 + nw],
                             start=False, stop=True)
        osb = outp.tile([128, DIM], BF16, tag="osb", name="osb")
        nc.scalar.copy(osb[:, 0:384], pp[:, 0:384])
        nc.vector.tensor_copy(osb[:, 384:768], pp[:, 384:768])
        eng = nc.sync if m % 2 == 0 else nc.scalar
        eng.dma_start(out_d[m * 128:(m + 1) * 128, :], osb[:])

    for kb in (5, 6, 7):
        AV_unit(HPC - 1, kb)
    for m in range(4):
        proj_t01(m)
    finish_head(HPC - 1, last=True)
    for m in range(4):
        proj_t2_out(m)
        proj_t01(m + 4)
    for m in range(4, 8):
        proj_t2_out(m)


def _host_prep(x, qkv_w, qkv_b, proj_w, proj_b, rel_pos_h, rel_pos_w):
    bf = ml_dtypes.bfloat16
    idx_h = np.arange(H)[:, None] - np.arange(H)[None, :] + (H - 1)
    idx_w = np.arange(W)[:, None] - np.arange(W)[None, :] + (W - 1)
    Rh = rel_pos_h[idx_h]  # [qh, kh, c]
    Rw = rel_pos_w[idx_w]  # [qw, kw, c]
    rhT8 = np.ascontiguousarray((8.0 * Rh).transpose(2, 0, 1).reshape(HD, H * H)).astype(bf)
    rwT8 = np.ascontiguousarray((8.0 * Rw).transpose(2, 0, 1).reshape(HD, W * W)).astype(bf)
    kt = np.arange(N)
    ec = np.zeros((64, N), np.float32)
    ec[:32] = (np.arange(32)[:, None] == (kt // 32)[None, :])
    ec[32:] = (np.arange(32)[:, None] == (kt % 32)[None, :])
    ec = ec.astype(bf)

    in_maps = []
    for core in range(NCORES):
        b = core // 2
        h0 = (core % 2) * HPC
        xTc = np.ascontiguousarray(x[b].reshape(N, DIM).T).astype(bf)
        wqkc = np.zeros((DIM, HPC * 128), np.float32)
        wvc = np.zeros((DIM, HPC * 64), np.float32)
        wpc = np.zeros((HPC * HD, DIM), np.float32)
        qkbc = np.zeros((128, HPC), np.float32)
        for h in range(HPC):
            g = h0 + h
            wqkc[:, h * 128:h * 128 + 64] = qkv_w[g * HD:(g + 1) * HD].T
            wqkc[:, h * 128 + 64:h * 128 + 128] = qkv_w[DIM + g * HD:DIM + (g + 1) * HD].T
            wvc[:, h * 64:(h + 1) * 64] = qkv_w[2 * DIM + g * HD:2 * DIM + (g + 1) * HD].T
            wpc[h * HD:(h + 1) * HD, :] = proj_w[:, g * HD:(g + 1) * HD].T
            qkbc[0:64, h] = qkv_b[g * HD:(g + 1) * HD] * 0.125
            qkbc[64:128, h] = qkv_b[DIM + g * HD:DIM + (g + 1) * HD]
        in_maps.append({
            "xT": xTc, "wqk": wqkc.astype(bf), "wv": wvc.astype(bf),
            "wp": wpc.astype(bf), "rhT": rhT8, "rwT": rwT8, "ecomb": ec,
            "qkb": qkbc,
        })
    return in_maps


def kernel(x, qkv_w, qkv_b, proj_w, proj_b, rel_pos_h, rel_pos_w, _trace=False):
    x = np.asarray(x, np.float32)
    qkv_w = np.asarray(qkv_w, np.float32)
    qkv_b = np.asarray(qkv_b, np.float32)
    proj_w = np.asarray(proj_w, np.float32)
    proj_b = np.asarray(proj_b, np.float32)
    rel_pos_h = np.asarray(rel_pos_h, np.float32)
    rel_pos_w = np.asarray(rel_pos_w, np.float32)

    in_maps = _host_prep(x, qkv_w, qkv_b, proj_w, proj_b, rel_pos_h, rel_pos_w)
    if "nc" not in _cache:
        _cache["nc"] = build_program()
    nc = _cache["nc"]
    res = run_bass_kernel_spmd(nc, in_maps, core_ids=list(range(NCORES)),
                               trace=_trace)
    parts = [np.asarray(r["out_part"], np.float32) for r in res.results]
    # v-bias enters the output as a constant row: bv @ proj_w.T (attn rows sum
    # to one), folded here together with proj_b.
    bias_row = proj_b + qkv_b[2 * DIM:] @ proj_w.T
    out = np.zeros((B, N, DIM), np.float32)
    for b in range(B):
        out[b] = parts[2 * b] + parts[2 * b + 1] + bias_row
    if _trace:
        kernel.last_results = res
    return out.reshape(B, H, W, DIM)
